# revision 1
# baseline (speedup 1.0000x reference)
"""Distributed Trainium2 kernel for nn_Attention_21990232555717.

Reference (per batch element a, seq b=1024, model dim c=1024, 16 heads):
    qkv = x @ w_qkv                       # (b, 3072)
    q,k,v split per head (hd=64)
    scores = q @ k.T * (1/sqrt(1024))     # (h, b, b)
    attn = softmax(scores, axis=HEADS)    # normalize across the 16 heads!
    out = attn @ v -> (b, 1024) @ w_out + b_out

Sharding: pure data parallel - batch (8) across 8 cores, weights replicated.
No collectives needed.

Per-core dataflow (f32r matmuls for projections, bf16 for the softmax path,
f32 accumulation in PSUM everywhere):
  xT   (c, s) f32r  from PE transposes of x
  QKT  (f, s) bf16  = w_qk^T @ x^T  (lhsT=w_qk f32r, rhs=xT f32r)
  Vb   (s, f) bf16  = x @ w_v       (lhsT=xT, rhs=w_v)
  scoresT (k, q) psum f32 per head  (lhsT=KT_h bf16, rhs=QT_h bf16)
  E = exp(scores/32) bf16; denom = sum_h E; attn = E * recip(denom)  [in-place]
  outT (f=h*64+d, q) f32r = accum_k (lhsT=Vb_h bf16, rhs=attn_h bf16)
  y (s, e) = (lhsT=outT f32r, rhs=w_out f32r) + ones^T b_out
"""

import numpy as np

import concourse.bass as bass
import concourse.mybir as mybir
import concourse.tile as tile
from concourse import bacc
from concourse.bass_utils import run_bass_kernel_spmd
from concourse.masks import make_identity

F32 = mybir.dt.float32
F32R = mybir.dt.float32r
BF16 = mybir.dt.bfloat16
Exp = mybir.ActivationFunctionType.Exp

S = 1024      # sequence length per core (batch element)
C = 1024      # model dim
H = 16        # heads
HD = 64       # head dim
SCALE = 1.0 / (C ** 0.5)
QB = 256      # q block size
NQB = S // QB          # 4 q blocks
NKT = S // 128         # 8 k tiles
NCT = C // 128         # 8 contraction tiles
NG = 4                 # k-tile groups of 2 per q block


def build():
    nc = bacc.Bacc(None, target_bir_lowering=False)
    x_ext = nc.declare_dram_parameter("x", [S, C], F32, isOutput=False)
    wqkv_ext = nc.declare_dram_parameter("w_qkv", [C, 3 * C], F32, isOutput=False)
    wout_ext = nc.declare_dram_parameter("w_out", [C, C], F32, isOutput=False)
    b_ext = nc.declare_dram_parameter("b_out", [C], F32, isOutput=False)
    out_ext = nc.declare_dram_parameter("out", [S, C], F32, isOutput=True)

    wqkv_r = wqkv_ext[:].bitcast(F32R)
    wout_r = wout_ext[:].bitcast(F32R)

    with tile.TileContext(nc) as tc:
        with (
            tc.tile_pool(name="const_p", bufs=1) as const_p,
            tc.tile_pool(name="act_p", bufs=1) as act_p,
        ):
            # ---- constants ----
            ident = const_p.tile([128, 128], F32)
            make_identity(nc, ident)
            ones1 = const_p.tile([1, 128], BF16)
            nc.vector.memset(ones1, 1.0)
            b_f = const_p.tile([1, C], F32)
            nc.sync.dma_start(b_f, b_ext[None, :])
            b_sb = const_p.tile([1, C], BF16)
            nc.vector.tensor_copy(b_sb, b_f)

            # ---- persistent activations ----
            QKT = act_p.tile([128, H, S], BF16)        # 4 MB  (Q tiles 0..7, K tiles 8..15)
            Vb = act_p.tile([128, NKT, C], BF16)       # 2 MB

            # ============ stages A-C: transpose x, qkv projections ============
            with (
                tc.tile_pool(name="ps_t", bufs=2, space="PSUM") as ps_t,
                tc.tile_pool(name="ps_b", bufs=2, space="PSUM") as ps_b,
                tc.tile_pool(name="xt_p", bufs=1) as xt_p,
                tc.tile_pool(name="xs_p", bufs=2) as xs_p,
                tc.tile_pool(name="w_p", bufs=1) as w_p,
            ):
                xT = xt_p.tile([128, NCT, S], F32R)        # 4 MB
                wqk = w_p.tile([128, NCT, 2 * C], F32R)    # 8 MB
                wv = w_p.tile([128, NCT, C], F32R)         # 4 MB

                with nc.named_scope("transpose_x"):
                    for st in range(NKT):
                        xs = xs_p.tile([128, C], F32, tag="xslab")
                        nc.sync.dma_start(xs, x_ext[st * 128:(st + 1) * 128, :])
                        for ct in range(NCT):
                            pt = ps_t.tile([128, 128], F32)
                            nc.tensor.transpose(pt, xs[:, ct * 128:(ct + 1) * 128], ident)
                            if ct % 2 == 0:
                                nc.vector.tensor_copy(xT[:, ct, st * 128:(st + 1) * 128], pt)
                            else:
                                nc.scalar.copy(xT[:, ct, st * 128:(st + 1) * 128], pt)

                with nc.named_scope("load_weights"):
                    for ct in range(NCT):
                        nc.sync.dma_start(
                            wqk[:, ct, :], wqkv_r[ct * 128:(ct + 1) * 128, 0:2 * C])
                    for ct in range(NCT):
                        nc.sync.dma_start(
                            wv[:, ct, :], wqkv_r[ct * 128:(ct + 1) * 128, 2 * C:3 * C])

                # ---- stage B: QKT = w_qk^T @ x^T ----
                with nc.named_scope("qk_proj"):
                    for ft in range(H):
                        pss = [ps_b.tile([128, 512], F32, tag=f"psb{sb}", name=f"psb{ft}_{sb}")
                               for sb in range(2)]
                        for ct in range(NCT):
                            lhsT = wqk[:, ct, ft * 128:(ft + 1) * 128]
                            for sb in range(2):
                                nc.tensor.matmul(
                                    pss[sb], lhsT, xT[:, ct, sb * 512:(sb + 1) * 512],
                                    start=(ct == 0), stop=(ct == NCT - 1),
                                )
                        for sb in range(2):
                            if ft % 2 == 0:
                                nc.scalar.copy(QKT[:, ft, sb * 512:(sb + 1) * 512], pss[sb])
                            else:
                                nc.vector.tensor_copy(QKT[:, ft, sb * 512:(sb + 1) * 512], pss[sb])

                # ---- stage C: Vb = x @ w_v ----
                with nc.named_scope("v_proj"):
                    for st in range(NKT):
                        pss = [ps_b.tile([128, 512], F32, tag=f"psb{fb}", name=f"psc{st}_{fb}")
                               for fb in range(2)]
                        for ct in range(NCT):
                            lhsT = xT[:, ct, st * 128:(st + 1) * 128]
                            for fb in range(2):
                                nc.tensor.matmul(
                                    pss[fb], lhsT, wv[:, ct, fb * 512:(fb + 1) * 512],
                                    start=(ct == 0), stop=(ct == NCT - 1),
                                )
                        for fb in range(2):
                            if st % 2 == 0:
                                nc.scalar.copy(Vb[:, st, fb * 512:(fb + 1) * 512], pss[fb])
                            else:
                                nc.vector.tensor_copy(Vb[:, st, fb * 512:(fb + 1) * 512], pss[fb])

            # ================= stage D/E: attention + out proj =================
            with (
                tc.tile_pool(name="ps_s", bufs=2, space="PSUM") as ps_s,
                tc.tile_pool(name="ps_o", bufs=1, space="PSUM") as ps_o,
                tc.tile_pool(name="ps_y", bufs=2, space="PSUM") as ps_y,
                tc.tile_pool(name="wout_p", bufs=1) as wout_p,
                tc.tile_pool(name="e_pool", bufs=1) as e_pool,
                tc.tile_pool(name="d_pool", bufs=1) as d_pool,
                tc.tile_pool(name="o_pool", bufs=2) as o_pool,
                tc.tile_pool(name="y_pool", bufs=2) as y_pool,
            ):
                wout = wout_p.tile([128, NCT, C], BF16)    # 2 MB
                with nc.named_scope("load_wout"):
                    for ft in range(NCT):
                        wt = y_pool.tile([128, C], F32, tag="wtmp", name=f"wt{ft}", bufs=2)
                        nc.sync.dma_start(wt, wout_ext[ft * 128:(ft + 1) * 128, :])
                        nc.vector.tensor_copy(wout[:, ft, :], wt)

                def emit_out_proj(q0, outT):
                    with nc.named_scope(f"out_proj_q{q0}"):
                        for qsub in range(QB // 128):
                            for ec in range(2):
                                psy = ps_y.tile([128, 512], F32, tag="psy",
                                                name=f"psy{q0}_{qsub}_{ec}")
                                for ft in range(NCT):
                                    nc.tensor.matmul(
                                        psy,
                                        outT[:, ft, qsub * 128:(qsub + 1) * 128],
                                        wout[:, ft, ec * 512:(ec + 1) * 512],
                                        start=(ft == 0), stop=False,
                                    )
                                nc.tensor.matmul(
                                    psy, ones1, b_sb[:, ec * 512:(ec + 1) * 512],
                                    start=False, stop=True,
                                )
                                y = y_pool.tile([128, 512], F32, tag="y",
                                                name=f"y{q0}_{qsub}_{ec}")
                                nc.scalar.copy(y, psy)
                                nc.sync.dma_start(
                                    out_ext[q0 + qsub * 128:q0 + (qsub + 1) * 128,
                                            ec * 512:(ec + 1) * 512],
                                    y,
                                )

                pending = None  # deferred out-proj: (q0, outT)
                for qb in range(NQB):
                    q0 = qb * QB
                    Etiles = {}
                    with nc.named_scope(f"attn_qb{qb}"):
                        for gg in range(2):  # groups of 4 k-tiles
                            # ---- D1: scores + exp (one wide ACT op per head) ----
                            Eev = e_pool.tile([128, 8, 4 * QB], BF16, tag=f"Eev{gg}",
                                              name=f"Eev{gg}")
                            Eod = e_pool.tile([128, 8, 4 * QB], BF16, tag=f"Eod{gg}",
                                              name=f"Eod{gg}")
                            pss = ps_s.tile([128, 4 * QB], F32, tag="scores",
                                            name=f"sc{qb}_{gg}_0")
                            for h in range(H):
                                po = 64 * (h % 2)
                                rhs = QKT[po:po + 64, h // 2, q0:q0 + QB]
                                for j in range(4):
                                    kt = 4 * gg + j
                                    lhsT = QKT[po:po + 64, 8 + h // 2, kt * 128:(kt + 1) * 128]
                                    nc.tensor.matmul(pss[:, j * QB:(j + 1) * QB], lhsT, rhs,
                                                     start=True, stop=True)
                                et = (Eev if h % 2 == 0 else Eod)[:, h // 2, :]
                                nc.scalar.activation(et, pss, Exp, scale=SCALE)
                                Etiles[(h, gg)] = et
                                if h < H - 1:
                                    pss = ps_s.tile([128, 4 * QB], F32, tag="scores",
                                                    name=f"sc{qb}_{gg}_{h + 1}")
                            if pending is not None and gg == 0:
                                # slot previous block's out-proj here so the PE has
                                # dense work while D2 runs on DVE/GPSIMD
                                emit_out_proj(*pending)
                                pending = None
                            # ---- D2: denominator (wide flat tree) + normalize ----
                            dl1 = d_pool.tile([128, 8 * 4 * QB], BF16, tag="dl1",
                                              name="dl1", bufs=1)
                            nc.vector.tensor_add(
                                dl1, Eev.rearrange("p a b -> p (a b)"),
                                Eod.rearrange("p a b -> p (a b)"))
                            dl2 = d_pool.tile([128, 4 * 4 * QB], BF16, tag="dl2",
                                              name="dl2", bufs=1)
                            nc.vector.tensor_add(dl2, dl1[:, 0:4 * 4 * QB],
                                                 dl1[:, 4 * 4 * QB:8 * 4 * QB])
                            dl3 = d_pool.tile([128, 2 * 4 * QB], BF16, tag="dl3",
                                              name="dl3", bufs=1)
                            nc.vector.tensor_add(dl3, dl2[:, 0:2 * 4 * QB],
                                                 dl2[:, 2 * 4 * QB:4 * 4 * QB])
                            denf = d_pool.tile([128, 4 * QB], F32, tag="denf",
                                               name="denf", bufs=1)
                            nc.vector.tensor_add(denf, dl3[:, 0:4 * QB],
                                                 dl3[:, 4 * QB:2 * 4 * QB])
                            rec_f = d_pool.tile([128, 4 * QB], F32, tag="recf", bufs=1)
                            nc.vector.reciprocal_approx_fast(out=rec_f, in_=denf)
                            rec = d_pool.tile([128, 4 * QB], BF16, tag="rec", bufs=2)
                            nc.vector.tensor_copy(rec, rec_f)
                            for h in range(H):
                                et = Etiles[(h, gg)]
                                nc.vector.tensor_mul(et, et, rec)
                        # ---- D3: attn @ v in 8 waves of 2 heads ----
                        # one psum bank per head per wave: a single accumulation
                        # group per 2KB zero region (start=True zeroes the whole
                        # region, so interleaved per-head groups in one bank
                        # would corrupt each other)
                        outT = o_pool.tile([128, NCT, QB], BF16, tag="outT",
                                           name=f"outT{qb}")
                        for w in range(NKT):
                            aw = ps_o.tile([128, 2, 512], F32, tag="acc",
                                           name=f"acc{qb}_{w}")
                            for kt in range(NKT):
                                gg, j = kt // 4, kt % 4
                                for i in range(2):
                                    h = 2 * w + i
                                    po = 64 * (h % 2)
                                    nc.tensor.matmul(
                                        aw[po:po + 64, i, 0:QB],
                                        Vb[:, kt, h * HD:(h + 1) * HD],
                                        Etiles[(h, gg)][:, j * QB:(j + 1) * QB],
                                        # (slice of paired big E tile)
                                        start=(kt == 0), stop=(kt == NKT - 1),
                                        tile_position=(0, po),
                                    )
                            for i in range(2):
                                h = 2 * w + i
                                po = 64 * (h % 2)
                                if i % 2 == 0:
                                    nc.vector.tensor_copy(
                                        outT[po:po + 64, h // 2, :], aw[po:po + 64, i, 0:QB])
                                else:
                                    nc.scalar.copy(
                                        outT[po:po + 64, h // 2, :], aw[po:po + 64, i, 0:QB])
                    pending = (q0, outT)
                emit_out_proj(*pending)

    nc.compile()
    return nc


_NC = None


def _get_nc():
    global _NC
    if _NC is None:
        _NC = build()
    return _NC


def kernel(x, w_qkv, w_out, b_out):
    nc = _get_nc()
    x = np.ascontiguousarray(np.asarray(x, dtype=np.float32))
    w_qkv = np.ascontiguousarray(np.asarray(w_qkv, dtype=np.float32))
    w_out = np.ascontiguousarray(np.asarray(w_out, dtype=np.float32))
    b_out = np.ascontiguousarray(np.asarray(b_out, dtype=np.float32))
    in_maps = [
        {"x": x[i], "w_qkv": w_qkv, "w_out": w_out, "b_out": b_out}
        for i in range(8)
    ]
    res = run_bass_kernel_spmd(nc, in_maps, core_ids=list(range(8)))
    out = np.stack([np.asarray(res.results[i]["out"]) for i in range(8)])
    return out.astype(np.float32)



# revision 7
# speedup vs baseline: 1.2718x; 1.2718x over previous
"""Distributed Trainium2 kernel for nn_Attention_21990232555717.

Reference (per batch element a, seq s=1024, model dim c=1024, 16 heads):
    qkv = x @ w_qkv                       # (s, 3072)
    scores = q @ k.T * (1/sqrt(1024))     # (h, s, s)
    attn = softmax(scores, axis=HEADS)    # normalize across the 16 heads
    out = attn @ v -> (s, 1024) @ w_out + b_out

Sharding: pure data parallel - batch (8) across 8 cores, weights replicated.

Per-core dataflow, all bf16 on the matmul paths (inputs converted to bf16
on the host, f32 accumulation in PSUM):
  xT   (c, s)  via DMA-XBAR transpose straight from DRAM (no PE transposes)
  QKT  (f, s)  = w^T @ x^T    Q tiles 0..7, K tiles 8..15 (2 heads/tile)
  Vb   (s, f)  = x @ w_v
  per q-block of 128 (8 blocks), per head: scoresT (k,q) in PSUM
    E = exp(scores/32) bf16; D = sum_h E (incremental pair adds + folds);
    attn = E * recip(D)  [in-place]
  outT (f, q) = accum_k V_h^T-slices @ attn_h   (2 heads packed per matmul
    via column groups)
  y (q, e) = outT^T @ w_out + ones^T b_out, DMA'd out per q-block

The emission order software-pipelines everything: K/Q projection tiles are
interleaved with q-block-0 score pairs so the scalar engine starts exp'ing
~20us in; attnV waves / out-proj / V-proj chunks are emitted as PE "filler"
between later score pairs so the PE never idles long enough to lose the
HAM 2.4GHz clock.
"""

from collections import deque

import numpy as np
import ml_dtypes

import concourse.bass as bass
import concourse.mybir as mybir
import concourse.tile as tile
from concourse import bacc
from concourse.bass_utils import run_bass_kernel_spmd

F32 = mybir.dt.float32
BF16 = mybir.dt.bfloat16
Exp = mybir.ActivationFunctionType.Exp

S = 1024      # sequence length per core (batch element)
C = 1024      # model dim
H = 16        # heads
HD = 64       # head dim
SCALE = 1.0 / (C ** 0.5)
QB = 128      # q block size
NQB = S // QB          # 8 q blocks
NKT = S // 128         # 8 k tiles
NCT = C // 128         # 8 contraction tiles
NHP = H // 2           # 8 head pairs


def build():
    nc = bacc.Bacc(None, target_bir_lowering=False)
    x_ext = nc.declare_dram_parameter("x", [S, C], BF16, isOutput=False)
    wq_ext = nc.declare_dram_parameter("w_q", [C, C], BF16, isOutput=False)
    wk_ext = nc.declare_dram_parameter("w_k", [C, C], BF16, isOutput=False)
    wv_ext = nc.declare_dram_parameter("w_v", [C, C], BF16, isOutput=False)
    wout_ext = nc.declare_dram_parameter("w_out", [C, C], BF16, isOutput=False)
    b_ext = nc.declare_dram_parameter("b_out", [C], F32, isOutput=False)
    out_ext = nc.declare_dram_parameter("out", [S, C], F32, isOutput=True)

    with tile.TileContext(nc) as tc:
        with (
            tc.tile_pool(name="const_p", bufs=1) as const_p,
            tc.tile_pool(name="persist", bufs=1) as persist,
            tc.tile_pool(name="e_pool", bufs=2) as e_pool,
            tc.tile_pool(name="tmp_p", bufs=1) as tmp_p,
            tc.tile_pool(name="o_pool", bufs=2) as o_pool,
            tc.tile_pool(name="ps_sc", bufs=2, space="PSUM") as ps_sc,
            tc.tile_pool(name="ps_big", bufs=2, space="PSUM") as ps_big,
        ):
            # ---- constants + ACT exp-table warm ----
            ones1 = const_p.tile([1, 128], BF16)
            nc.vector.memset(ones1, 1.0)
            dum = const_p.tile([1, 128], BF16)
            nc.scalar.activation(dum, ones1, Exp)  # pull ACT_TABLE_LOAD to t=0
            b_f = const_p.tile([1, C], F32)
            nc.sync.dma_start(b_f, b_ext[None, :])
            b_sb = const_p.tile([1, C], BF16)
            nc.vector.tensor_copy(b_sb, b_f)

            # ---- persistent activations ----
            xT = persist.tile([128, NCT, S], BF16)      # 16 KB/part
            QKT = persist.tile([128, H, S], BF16)       # 32 KB/part
            wv_sb = persist.tile([128, NCT, C], BF16)   # 16 KB/part

            # x transposed straight from DRAM through the DMA XBAR
            for ct in range(NCT):
                nc.sync.dma_start(
                    xT[:, ct, :], x_ext[:, ct * 128:(ct + 1) * 128],
                    transpose=True)
            for ct in range(NCT):
                nc.sync.dma_start(
                    wv_sb[:, ct, :], wv_ext[ct * 128:(ct + 1) * 128, :])

            # ---------------- helpers ----------------
            def vcopy(dst, src):
                nc.vector.tensor_copy(dst, src)

            def scopy(dst, src):
                nc.scalar.copy(dst, src)

            def proj_tile(dst, w_sb, ft, eng):
                """QKT[:, dst, :] = (x @ w[:, ft-tile])^T, one 128-row tile."""
                pss = ps_big.tile([128, 2, 512], F32, tag="big",
                                  name=f"pj{dst}")
                for ct in range(NCT):
                    lhsT = w_sb[:, ct, ft * 128:(ft + 1) * 128]
                    for sb in range(2):
                        nc.tensor.matmul(
                            pss[:, sb, :], lhsT,
                            xT[:, ct, sb * 512:(sb + 1) * 512],
                            start=(ct == 0), stop=(ct == NCT - 1))
                for sb in range(2):
                    eng(QKT[:, dst, sb * 512:(sb + 1) * 512], pss[:, sb, :])

            def score_pair(qb, hp, Eev_t, Eod_t, p_t):
                """scores + exp for heads (2hp, 2hp+1) of q-block qb, plus
                the incremental denominator pair-add on DVE."""
                pss_e = ps_sc.tile([128, S], F32, tag="sc",
                                   name=f"sc{qb}_{hp}e")
                pss_o = ps_sc.tile([128, S], F32, tag="sc",
                                   name=f"sc{qb}_{hp}o")
                for kt in range(NKT):
                    for po, pss in ((0, pss_e), (64, pss_o)):
                        nc.tensor.matmul(
                            pss[:, kt * 128:(kt + 1) * 128],
                            QKT[po:po + 64, 8 + hp, kt * 128:(kt + 1) * 128],
                            QKT[po:po + 64, hp, qb * QB:(qb + 1) * QB],
                            start=True, stop=True)
                nc.scalar.activation(Eev_t[:, hp, :], pss_e, Exp, scale=SCALE)
                nc.scalar.activation(Eod_t[:, hp, :], pss_o, Exp, scale=SCALE)
                nc.vector.tensor_add(p_t[:, hp, :], Eev_t[:, hp, :],
                                     Eod_t[:, hp, :])

            def denom_norm(qb, Eev_t, Eod_t, p_t):
                """fold pair-sums -> D, rec = 1/D, normalize E in place."""
                nc.vector.tensor_add(p_t[:, 0:4, :], p_t[:, 0:4, :],
                                     p_t[:, 4:8, :])
                nc.vector.tensor_add(p_t[:, 0:2, :], p_t[:, 0:2, :],
                                     p_t[:, 2:4, :])
                denf = tmp_p.tile([128, S], F32, tag="denf", name=f"denf{qb}")
                nc.vector.tensor_add(denf, p_t[:, 0, :], p_t[:, 1, :])
                recf = tmp_p.tile([128, S], F32, tag="recf", name=f"recf{qb}")
                nc.vector.reciprocal_approx_fast(out=recf, in_=denf)
                rec = tmp_p.tile([128, S], BF16, tag="rec", name=f"rec{qb}")
                nc.vector.tensor_copy(rec, recf)
                for hp in range(NHP):
                    nc.vector.tensor_mul(Eev_t[:, hp, :], Eev_t[:, hp, :], rec)
                    nc.vector.tensor_mul(Eod_t[:, hp, :], Eod_t[:, hp, :], rec)

            def attnv_wave(qb, w, Eev_t, Eod_t, outT_t):
                """attn @ v for heads (2w, 2w+1), packed via column groups."""
                aw = ps_big.tile([128, 2, 512], F32, tag="big",
                                 name=f"aw{qb}_{w}")
                for kt in range(NKT):
                    for i in (0, 1):
                        h = 2 * w + i
                        po = 64 * i
                        et = Eev_t if i == 0 else Eod_t
                        nc.tensor.matmul(
                            aw[po:po + 64, i, 0:QB],
                            Vb[:, kt, h * HD:(h + 1) * HD],
                            et[:, w, kt * 128:(kt + 1) * 128],
                            start=(kt == 0), stop=(kt == NKT - 1),
                            tile_position=(0, po))
                if w % 2 == 0:
                    nc.vector.tensor_copy(outT_t[0:64, w, :], aw[0:64, 0, 0:QB])
                    nc.scalar.copy(outT_t[64:128, w, :], aw[64:128, 1, 0:QB])
                else:
                    nc.scalar.copy(outT_t[0:64, w, :], aw[0:64, 0, 0:QB])
                    nc.vector.tensor_copy(outT_t[64:128, w, :],
                                          aw[64:128, 1, 0:QB])

            def out_proj(qb, outT_t):
                psy = ps_big.tile([128, 2, 512], F32, tag="big",
                                  name=f"psy{qb}")
                for ec in range(2):
                    for ft in range(NCT):
                        nc.tensor.matmul(
                            psy[:, ec, :], outT_t[:, ft, :],
                            wout_sb[:, ft, ec * 512:(ec + 1) * 512],
                            start=(ft == 0), stop=False)
                    nc.tensor.matmul(
                        psy[:, ec, :], ones1, b_sb[:, ec * 512:(ec + 1) * 512],
                        start=False, stop=True)
                y_t = o_pool.tile([128, C], F32, tag="y", name=f"y{qb}")
                nc.scalar.copy(y_t[:, 0:512], psy[:, 0, :])
                nc.scalar.copy(y_t[:, 512:1024], psy[:, 1, :])
                nc.sync.dma_start(out_ext[qb * QB:(qb + 1) * QB, :], y_t)

            def v_chunk(st, eng0, eng1):
                """Vb[:, st, :] = x-rows-st @ w_v  (one 128-row slab)."""
                pss = ps_big.tile([128, 2, 512], F32, tag="big",
                                  name=f"vp{st}")
                for ct in range(NCT):
                    lhsT = xT[:, ct, st * 128:(st + 1) * 128]
                    for fb in range(2):
                        nc.tensor.matmul(
                            pss[:, fb, :], lhsT,
                            wv_sb[:, ct, fb * 512:(fb + 1) * 512],
                            start=(ct == 0), stop=(ct == NCT - 1))
                eng0(Vb[:, st, 0:512], pss[:, 0, :])
                eng1(Vb[:, st, 512:1024], pss[:, 1, :])

            def new_E(qb):
                Eev_t = e_pool.tile([128, NHP, S], BF16, tag="Eev",
                                    name=f"Eev{qb}")
                Eod_t = e_pool.tile([128, NHP, S], BF16, tag="Eod",
                                    name=f"Eod{qb}")
                p_t = tmp_p.tile([128, NHP, S], BF16, tag="p", name=f"p{qb}")
                return Eev_t, Eod_t, p_t

            # ============ lead-in: K/Q projection ∥ q-block 0 ============
            E0 = new_E(0)
            with tc.tile_pool(name="wkq_p", bufs=1) as wkq_p:
                wk_sb = wkq_p.tile([128, NCT, C], BF16)
                wq_sb = wkq_p.tile([128, NCT, C], BF16)
                for ct in range(NCT):
                    nc.sync.dma_start(
                        wk_sb[:, ct, :], wk_ext[ct * 128:(ct + 1) * 128, :])
                for ct in range(NCT):
                    nc.sync.dma_start(
                        wq_sb[:, ct, :], wq_ext[ct * 128:(ct + 1) * 128, :])
                with nc.named_scope("kq_proj"):
                    for ft in range(NHP):
                        proj_tile(8 + ft, wk_sb, ft, vcopy)
                        proj_tile(ft, wq_sb, ft, scopy)
                        score_pair(0, ft, *E0)

            # ============ main: q-blocks 1..7 with PE fillers ============
            with tc.tile_pool(name="rest_p", bufs=1) as rest_p:
                Vb = rest_p.tile([128, NKT, C], BF16)
                wout_sb = rest_p.tile([128, NCT, C], BF16)
                for ft in range(NCT):
                    nc.sync.dma_start(
                        wout_sb[:, ft, :], wout_ext[ft * 128:(ft + 1) * 128, :])

                denom_norm(0, *E0)

                fillers = deque()
                for st in range(NKT):
                    fillers.append(
                        lambda st=st: v_chunk(
                            st, vcopy if st % 2 else scopy,
                            scopy if st % 2 else vcopy))
                outT0 = o_pool.tile([128, NCT, QB], BF16, tag="outT",
                                    name="outT0")
                for w in range(NHP):
                    fillers.append(
                        lambda w=w: attnv_wave(0, w, E0[0], E0[1], outT0))
                fillers.append(lambda: out_proj(0, outT0))

                prev = (E0, outT0)
                for qb in range(1, NQB):
                    Eq = new_E(qb)
                    with nc.named_scope(f"qb{qb}"):
                        for hp in range(NHP):
                            score_pair(qb, hp, *Eq)
                            for _ in range(2):
                                if fillers:
                                    fillers.popleft()()
                        while fillers:
                            fillers.popleft()()
                        denom_norm(qb, *Eq)
                    outT_t = o_pool.tile([128, NCT, QB], BF16, tag="outT",
                                         name=f"outT{qb}")
                    for w in range(NHP):
                        fillers.append(
                            lambda w=w, Eq=Eq, o=outT_t, qb=qb:
                            attnv_wave(qb, w, Eq[0], Eq[1], o))
                    fillers.append(
                        lambda qb=qb, o=outT_t: out_proj(qb, o))
                    prev = (Eq, outT_t)

                with nc.named_scope("tail"):
                    while fillers:
                        fillers.popleft()()

    nc.compile()
    return nc


_NC = None


def _get_nc():
    global _NC
    if _NC is None:
        _NC = build()
    return _NC


def make_in_maps(x, w_qkv, w_out, b_out):
    bf = ml_dtypes.bfloat16
    x = np.asarray(x, dtype=np.float32)
    w_qkv = np.asarray(w_qkv, dtype=np.float32)
    wq = np.ascontiguousarray(w_qkv[:, 0:C]).astype(bf)
    wk = np.ascontiguousarray(w_qkv[:, C:2 * C]).astype(bf)
    wv = np.ascontiguousarray(w_qkv[:, 2 * C:3 * C]).astype(bf)
    wo = np.ascontiguousarray(np.asarray(w_out, dtype=np.float32)).astype(bf)
    b = np.ascontiguousarray(np.asarray(b_out, dtype=np.float32))
    xb = x.astype(bf)
    return [
        {"x": np.ascontiguousarray(xb[i]), "w_q": wq, "w_k": wk, "w_v": wv,
         "w_out": wo, "b_out": b}
        for i in range(8)
    ]


def kernel(x, w_qkv, w_out, b_out):
    nc = _get_nc()
    in_maps = make_in_maps(x, w_qkv, w_out, b_out)
    res = run_bass_kernel_spmd(nc, in_maps, core_ids=list(range(8)))
    out = np.stack([np.asarray(res.results[i]["out"]) for i in range(8)])
    return out.astype(np.float32)
